# revision 8
# baseline (speedup 1.0000x reference)
"""ArcFace loss (margin softmax CE + logits) on 8 TRN2 NeuronCores.

Strategy: classifier/tensor parallel — W and all (N, classes) intermediates
are sharded along the classes axis across the 8 cores. The global softmax
denominator needs one 8-KB AllReduce of per-row sum-exp (we use a fixed
max shift of SCALE=64, valid because cos in [-1, 1], so no max AllReduce
is needed). The margin-modified target logit is computed redundantly on
every core from host-gathered W[label] rows, so it needs no collective.

Per-core device pipeline (Tile):
  - embT (512, N) f32 staged -> bf16 (TensorE stationary operands)
  - emb/wlab (N, 512) f32 streamed in chunks -> row dots + row sumsq
    (margin/target path, f32)
  - W^T shard (512, C) f32 staged by j-groups -> bf16; per-class sumsq via
    TensorE diag-blocks (W^T.T @ W^T masked by identity) -> 64/||w_j|| row
    -> partition-broadcast to a (128, C) column-scale tile
  - main loop: cos64 = (embT_bf.T @ wt_bf) * rnormE[i] * (64*rnormW[j]),
    copied PSUM->SBUF with both scales fused in one DVE op; DMA to out;
    ACT exp(x - 64) with per-row accumulation for the softmax denominator
  - epilogue: sum-exp correction for the margin target, one AllReduce,
    log-sum-exp, mean NLL -> loss scalar
"""

import math

import numpy as np

import concourse.bass as bass
import concourse.mybir as mybir
import concourse.tile as tile
from concourse import bacc
from concourse.masks import make_identity

N_CORES = 8
BATCH = 2048
EMB = 512
CLASSES = 50000
CPC = CLASSES // N_CORES  # 6250 classes per core

MARGIN = 0.6
SCALE = 64.0
EPS = 1e-7
COS_M = math.cos(MARGIN)
SIN_M = math.sin(MARGIN)
TH = math.cos(math.pi - MARGIN)
MM_ = math.sin(math.pi - MARGIN) * MARGIN

F32 = mybir.dt.float32
BF16 = mybir.dt.bfloat16
P = 128
JT = 512          # psum bank width in f32 (matmul max free dim)
GROUP_JT = 3      # j-tiles per pipeline group


def build_nc(batch=BATCH, cpc=CPC, emb=EMB, n_cores=N_CORES, use_bf16=True,
             debug_outs=False):
    KC = emb // P     # contraction chunks
    IC = batch // P   # batch row-chunks
    mm_dt = BF16 if use_bf16 else F32

    # j-groups: units of the load->normalize->matmul->store pipeline
    groups = []
    j = 0
    while j < cpc:
        w = min(GROUP_JT * JT, cpc - j)
        groups.append((j, w))
        j += w
    NG = len(groups)

    nc = bacc.Bacc("TRN2", target_bir_lowering=False, debug=False,
                   num_devices=n_cores)

    embT_d = nc.dram_tensor("embT", [emb, batch], F32, kind="ExternalInput")
    emb_d = nc.dram_tensor("embn", [batch, emb], F32, kind="ExternalInput")
    wlab_d = nc.dram_tensor("wlab", [batch, emb], F32, kind="ExternalInput")
    wt_d = nc.dram_tensor("wt", [emb, cpc], F32, kind="ExternalInput")
    out_d = nc.dram_tensor("out", [batch, cpc], F32, kind="ExternalOutput")
    loss_d = nc.dram_tensor("loss", [1, 1], F32, kind="ExternalOutput")
    dbg_d = None
    if debug_outs:
        IC_ = batch // P
        dbg_d = nc.dram_tensor("dbg", [P, IC_ * 6 + 1], F32, kind="ExternalOutput")

    AF = mybir.ActivationFunctionType
    OP = mybir.AluOpType

    with tile.TileContext(nc) as tc:
        with (
            tc.tile_pool(name="const", bufs=1) as constp,
            tc.tile_pool(name="stg", bufs=2) as stgp,
            tc.tile_pool(name="wbf", bufs=1) as wbfp,
            tc.tile_pool(name="ebf", bufs=1) as ebfp,
            tc.tile_pool(name="scale", bufs=1) as scalep,
            tc.tile_pool(name="prep", bufs=2) as prepp,
            tc.tile_pool(name="small", bufs=1) as smallp,
            tc.tile_pool(name="md", bufs=2) as mdp,
            tc.tile_pool(name="obuf", bufs=2) as obufp,
            tc.tile_pool(name="junk", bufs=2) as junkp,
            tc.tile_pool(name="pmm", bufs=2, space="PSUM") as pmmp,
            tc.tile_pool(name="pdg", bufs=1, space="PSUM") as pdgp,
            tc.tile_pool(name="prow", bufs=1, space="PSUM") as prowp,
            tc.tile_pool(name="dram", bufs=1, space="DRAM") as dramp,
        ):
            # ---- constants ----
            identity = constp.tile([P, P], F32)
            make_identity(nc, identity[:, :])
            ones = constp.tile([P, 1], F32)
            nc.vector.memset(ones[:, :], 1.0)
            bias_m64 = constp.tile([P, 1], F32)
            nc.vector.memset(bias_m64[:, :], -float(SCALE))

            # ---- persistent tiles ----
            wt_bf = wbfp.tile([P, KC, cpc], mm_dt)
            embT_bf = ebfp.tile([P, KC, batch], mm_dt)
            scale_bcast = scalep.tile([P, cpc], F32)
            expsums = smallp.tile([P, IC * NG], F32, tag="expsums")
            rawdot = smallp.tile([P, IC], F32, tag="rawdot")
            sumsqE = smallp.tile([P, IC], F32, tag="sumsqE")
            sumsqW = smallp.tile([P, IC], F32, tag="sumsqW")

            # ---- stage embT, convert to matmul dtype ----
            stg_e = stgp.tile([P, KC, batch], F32, tag="stg")
            for k in range(KC):
                nc.sync.dma_start(stg_e[:, k, :], embT_d[k * P:(k + 1) * P, :])
            for k in range(KC):
                nc.gpsimd.tensor_copy(embT_bf[:, k, :], stg_e[:, k, :])

            # ---- margin/target path: stream emb & wlab natural-layout ----
            junk_pr = smallp.tile([P, emb], F32, tag="junkpr")
            for c in range(IC):
                e_t = prepp.tile([P, emb], F32, tag="ept")
                w_t = prepp.tile([P, emb], F32, tag="wpt")
                nc.scalar.dma_start(e_t[:, :], emb_d[c * P:(c + 1) * P, :])
                nc.scalar.dma_start(w_t[:, :], wlab_d[c * P:(c + 1) * P, :])
                nc.vector.scalar_tensor_tensor(
                    junk_pr[:, :], e_t[:, :], 1.0, w_t[:, :],
                    op0=OP.mult, op1=OP.mult, accum_out=rawdot[:, c:c + 1])
                nc.vector.scalar_tensor_tensor(
                    junk_pr[:, :], e_t[:, :], 1.0, e_t[:, :],
                    op0=OP.mult, op1=OP.mult, accum_out=sumsqE[:, c:c + 1])
                nc.vector.scalar_tensor_tensor(
                    junk_pr[:, :], w_t[:, :], 1.0, w_t[:, :],
                    op0=OP.mult, op1=OP.mult, accum_out=sumsqW[:, c:c + 1])

            # rnorm = 1/||.|| per batch row (per-partition layout)
            tmpE = smallp.tile([P, IC], F32, tag="tmpE")
            rnormE = smallp.tile([P, IC], F32, tag="rnormE")
            nc.vector.reciprocal(tmpE[:, :], sumsqE[:, :])
            nc.scalar.activation(rnormE[:, :], tmpE[:, :], AF.Sqrt)
            tmpW = smallp.tile([P, IC], F32, tag="tmpW")
            rnormW = smallp.tile([P, IC], F32, tag="rnormW")
            nc.vector.reciprocal(tmpW[:, :], sumsqW[:, :])
            nc.scalar.activation(rnormW[:, :], tmpW[:, :], AF.Sqrt)

            # target cosine + ArcFace margin (all (P, IC), f32)
            ct = smallp.tile([P, IC], F32, tag="ct")
            nc.vector.tensor_mul(ct[:, :], rawdot[:, :], rnormE[:, :])
            nc.vector.tensor_mul(ct[:, :], ct[:, :], rnormW[:, :])
            nc.vector.tensor_scalar(ct[:, :], ct[:, :], 1.0 - EPS,
                                    -(1.0 - EPS), op0=OP.min, op1=OP.max)
            ct2 = smallp.tile([P, IC], F32, tag="ct2")
            nc.vector.tensor_mul(ct2[:, :], ct[:, :], ct[:, :])
            nc.vector.tensor_scalar(ct2[:, :], ct2[:, :], -1.0, 1.0,
                                    op0=OP.mult, op1=OP.add)  # 1 - ct^2
            sine = smallp.tile([P, IC], F32, tag="sine")
            nc.scalar.activation(sine[:, :], ct2[:, :], AF.Sqrt)
            nc.vector.tensor_scalar_max(sine[:, :], sine[:, :], EPS)
            t1 = smallp.tile([P, IC], F32, tag="t1")
            nc.vector.tensor_scalar_mul(t1[:, :], sine[:, :], SIN_M)
            phi = smallp.tile([P, IC], F32, tag="phi")
            nc.vector.scalar_tensor_tensor(phi[:, :], ct[:, :], COS_M, t1[:, :],
                                           op0=OP.mult, op1=OP.subtract)
            mask = smallp.tile([P, IC], F32, tag="mask")
            nc.vector.tensor_scalar(mask[:, :], ct[:, :], TH, None, op0=OP.is_gt)
            ctmm = smallp.tile([P, IC], F32, tag="ctmm")
            nc.vector.tensor_scalar(ctmm[:, :], ct[:, :], MM_, None,
                                    op0=OP.subtract)
            tl = smallp.tile([P, IC], F32, tag="tl")
            diff = smallp.tile([P, IC], F32, tag="diff")
            nc.vector.tensor_sub(diff[:, :], phi[:, :], ctmm[:, :])
            nc.vector.tensor_mul(diff[:, :], diff[:, :], mask[:, :])
            nc.vector.tensor_add(tl[:, :], ctmm[:, :], diff[:, :])
            # sum-exp correction: replace exp(64ct-64) by exp(64tl-64), /8
            ect = smallp.tile([P, IC], F32, tag="ect")
            nc.scalar.activation(ect[:, :], ct[:, :], AF.Exp,
                                 bias=bias_m64[:, :], scale=float(SCALE))
            etl = smallp.tile([P, IC], F32, tag="etl")
            nc.scalar.activation(etl[:, :], tl[:, :], AF.Exp,
                                 bias=bias_m64[:, :], scale=float(SCALE))
            corr = smallp.tile([P, IC], F32, tag="corr")
            nc.vector.tensor_sub(corr[:, :], ect[:, :], etl[:, :])

            # ---- W^T shard: per-group stage/normalize/matmul pipeline ----
            for g, (js, gw) in enumerate(groups):
                stg_w = stgp.tile([P, KC, gw], F32, tag="stg")
                for k in range(KC):
                    nc.sync.dma_start(stg_w[:, k, :],
                                      wt_d[k * P:(k + 1) * P, js:js + gw])
                for k in range(KC):
                    nc.gpsimd.tensor_copy(wt_bf[:, k, js:js + gw],
                                          stg_w[:, k, :])
                # per-class sumsq via diag of (W^T chunk).T @ (W^T chunk)
                nsq_g = junkp.tile([1, GROUP_JT * JT], F32, tag="nsqg")
                c0 = 0
                while c0 < gw:
                    cw = min(P, gw - c0)
                    pd = pdgp.tile([P, P], F32, tag="pd")
                    for k in range(KC):
                        nc.tensor.matmul(pd[:cw, :cw],
                                         stg_w[:, k, c0:c0 + cw],
                                         stg_w[:, k, c0:c0 + cw],
                                         start=(k == 0), stop=(k == KC - 1))
                    md = mdp.tile([P, P], F32, tag="md")
                    nc.vector.scalar_tensor_tensor(
                        md[:cw, :cw], pd[:cw, :cw], 1.0, identity[:cw, :cw],
                        op0=OP.mult, op1=OP.mult)
                    pr = prowp.tile([1, P], F32, tag="pr")
                    nc.tensor.matmul(pr[:1, :cw], ones[:cw, :], md[:cw, :cw],
                                     start=True, stop=True)
                    nc.scalar.activation(nsq_g[0:1, c0:c0 + cw],
                                         pr[:1, :cw], AF.Copy)
                    c0 += cw
                # 64 / ||w_j|| for the whole group, broadcast down partitions
                nc.vector.reciprocal(nsq_g[0:1, :gw], nsq_g[0:1, :gw])
                nc.scalar.activation(nsq_g[0:1, :gw], nsq_g[0:1, :gw], AF.Sqrt,
                                     scale=float(SCALE) * float(SCALE))
                nc.gpsimd.partition_broadcast(scale_bcast[:, js:js + gw],
                                              nsq_g[0:1, :gw])

                # ---- main loop for this group ----
                for ic in range(IC):
                    pt = pmmp.tile([P, GROUP_JT * JT], F32, tag="pt")
                    for k in range(KC):
                        a = 0
                        while a < gw:
                            w = min(JT, gw - a)
                            nc.tensor.matmul(
                                pt[:, a:a + w],
                                embT_bf[:, k, ic * P:(ic + 1) * P],
                                wt_bf[:, k, js + a:js + a + w],
                                start=(k == 0), stop=(k == KC - 1))
                            a += w
                    ob = obufp.tile([P, GROUP_JT * JT], F32, tag="ob")
                    nc.vector.scalar_tensor_tensor(
                        ob[:, :gw], pt[:, :gw], rnormE[:, ic:ic + 1],
                        scale_bcast[:, js:js + gw], op0=OP.mult, op1=OP.mult)
                    nc.sync.dma_start(out_d[ic * P:(ic + 1) * P, js:js + gw],
                                      ob[:, :gw])
                    je = junkp.tile([P, GROUP_JT * JT], F32, tag="je")
                    nc.scalar.activation(je[:, :gw], ob[:, :gw], AF.Exp,
                                         bias=bias_m64[:, :],
                                         accum_out=expsums[:, ic * NG + g:
                                                           ic * NG + g + 1])

            # ---- softmax denominator: local reduce, correct, AllReduce ----
            lsum = smallp.tile([P, IC], F32, tag="lsum")
            for ic in range(IC):
                nc.vector.tensor_reduce(lsum[:, ic:ic + 1],
                                        expsums[:, ic * NG:(ic + 1) * NG],
                                        axis=mybir.AxisListType.X, op=OP.add)
            lsumc = smallp.tile([P, IC], F32, tag="lsumc")
            nc.vector.scalar_tensor_tensor(lsumc[:, :], corr[:, :],
                                           -1.0 / n_cores, lsum[:, :],
                                           op0=OP.mult, op1=OP.add)

            cc_in = dramp.tile([P, IC], F32, tag="ccin")
            cc_out = dramp.tile([P, IC], F32, tag="ccout")
            nc.sync.dma_start(cc_in[:, :], lsumc[:, :])
            nc.gpsimd.collective_compute(
                "AllReduce", OP.add,
                replica_groups=[list(range(n_cores))],
                ins=[cc_in.opt()], outs=[cc_out.opt()])
            gsum = smallp.tile([P, IC], F32, tag="gsum")
            nc.sync.dma_start(gsum[:, :], cc_out[:, :])

            # loss = mean(64 + ln(gsum) - 64*tl)
            # Ln is inaccurate for tiny inputs: rescale into ~[1e-2, 1e2]
            # via ln(g) = ln(g * 2^64) - 64*ln2 (the -64*ln2 lands in the
            # final Copy bias below).
            lg = smallp.tile([P, IC], F32, tag="lg")
            nc.scalar.activation(lg[:, :], gsum[:, :], AF.Ln,
                                 scale=float(2.0 ** 64))
            nll = smallp.tile([P, IC], F32, tag="nll")
            nc.vector.scalar_tensor_tensor(nll[:, :], tl[:, :], -float(SCALE),
                                           lg[:, :], op0=OP.mult, op1=OP.add)
            nllr = smallp.tile([P, 1], F32, tag="nllr")
            nc.vector.tensor_reduce(nllr[:, :], nll[:, :],
                                    axis=mybir.AxisListType.X, op=OP.add)
            pl = prowp.tile([1, P], F32, tag="pr")
            nc.tensor.matmul(pl[:1, 0:1], ones[:, :], nllr[:, :],
                             start=True, stop=True)
            loss_sb = smallp.tile([1, 1], F32, tag="losssb")
            nc.scalar.activation(loss_sb[:, :], pl[:1, 0:1], AF.Copy,
                                 bias=float(SCALE) - 64.0 * math.log(2.0),
                                 scale=1.0 / batch)
            nc.sync.dma_start(loss_d[:, :], loss_sb[:, :])
            if debug_outs:
                nc.sync.dma_start(dbg_d[:, 0:IC], lsumc[:, :])
                nc.sync.dma_start(dbg_d[:, IC:2 * IC], gsum[:, :])
                nc.sync.dma_start(dbg_d[:, 2 * IC:3 * IC], ct[:, :])
                nc.sync.dma_start(dbg_d[:, 3 * IC:4 * IC], tl[:, :])
                nc.sync.dma_start(dbg_d[:, 4 * IC:5 * IC], lg[:, :])
                nc.sync.dma_start(dbg_d[:, 5 * IC:6 * IC], nll[:, :])
                nc.sync.dma_start(dbg_d[:, 6 * IC:6 * IC + 1], nllr[:, :])

    nc.compile()
    return nc


def _host_inputs(emb, label, W, cpc, n_cores):
    emb = np.ascontiguousarray(np.asarray(emb, dtype=np.float32))
    W = np.ascontiguousarray(np.asarray(W, dtype=np.float32))
    label = np.asarray(label).astype(np.int64)
    embT = np.ascontiguousarray(emb.T)
    wlab = np.ascontiguousarray(W[label])
    WT = np.ascontiguousarray(W.T)
    in_maps = []
    for d in range(n_cores):
        in_maps.append({
            "embT": embT,
            "embn": emb,
            "wlab": wlab,
            "wt": np.ascontiguousarray(WT[:, d * cpc:(d + 1) * cpc]),
        })
    return in_maps


_NC_CACHE = {}
LAST_RESULTS = None


def kernel(emb, label, W):
    global LAST_RESULTS
    from concourse.bass_utils import run_bass_kernel_spmd

    key = "full"
    if key not in _NC_CACHE:
        _NC_CACHE[key] = build_nc()
    nc = _NC_CACHE[key]

    in_maps = _host_inputs(emb, label, W, CPC, N_CORES)
    res = run_bass_kernel_spmd(nc, in_maps, core_ids=list(range(N_CORES)))
    LAST_RESULTS = res
    logits = np.concatenate([r["out"] for r in res.results], axis=1)
    loss = np.asarray(res.results[0]["loss"][0, 0], dtype=np.float32)
    return loss, logits


# revision 10
# speedup vs baseline: 1.1698x; 1.1698x over previous
"""ArcFace loss (margin softmax CE + logits) on 8 TRN2 NeuronCores.

Strategy: classifier/tensor parallel — W and all (N, classes) intermediates
are sharded along the classes axis across the 8 cores. The global softmax
denominator needs one 8-KB AllReduce of per-row sum-exp (we use a fixed
max shift of SCALE=64, valid because cos in [-1, 1], so no max AllReduce
is needed). The margin-modified target logit is computed redundantly on
every core from host-gathered W[label] rows, so it needs no collective.

Per-core device pipeline (Tile):
  - embT (512, N) f32 staged -> bf16 (TensorE stationary operands)
  - emb/wlab (N, 512) f32 streamed in chunks -> row dots + row sumsq
    (margin/target path, f32)
  - W^T shard (512, C) f32 staged by j-groups -> bf16; per-class sumsq via
    TensorE diag-blocks (W^T.T @ W^T masked by identity) -> 64/||w_j|| row
    -> partition-broadcast to a (128, C) column-scale tile
  - main loop: cos64 = (embT_bf.T @ wt_bf) * rnormE[i] * (64*rnormW[j]),
    copied PSUM->SBUF with both scales fused in one DVE op; DMA to out;
    ACT exp(x - 64) with per-row accumulation for the softmax denominator
  - epilogue: sum-exp correction for the margin target, one AllReduce,
    log-sum-exp, mean NLL -> loss scalar
"""

import math

import numpy as np

import concourse.bass as bass
import concourse.mybir as mybir
import concourse.tile as tile
from concourse import bacc
from concourse.masks import make_identity

N_CORES = 8
BATCH = 2048
EMB = 512
CLASSES = 50000
CPC = CLASSES // N_CORES  # 6250 classes per core

MARGIN = 0.6
SCALE = 64.0
EPS = 1e-7
COS_M = math.cos(MARGIN)
SIN_M = math.sin(MARGIN)
TH = math.cos(math.pi - MARGIN)
MM_ = math.sin(math.pi - MARGIN) * MARGIN

F32 = mybir.dt.float32
BF16 = mybir.dt.bfloat16
P = 128
JT = 512          # psum bank width in f32 (matmul max free dim)
GROUP_JT = 3      # j-tiles per pipeline group


def build_nc(batch=BATCH, cpc=CPC, emb=EMB, n_cores=N_CORES, use_bf16=True,
             debug_outs=False):
    KC = emb // P     # contraction chunks
    IC = batch // P   # batch row-chunks
    mm_dt = BF16 if use_bf16 else F32

    # j-groups: units of the load->normalize->matmul->store pipeline
    groups = []
    j = 0
    while j < cpc:
        w = min(GROUP_JT * JT, cpc - j)
        groups.append((j, w))
        j += w
    NG = len(groups)

    nc = bacc.Bacc("TRN2", target_bir_lowering=False, debug=False,
                   num_devices=n_cores)

    embT_d = nc.dram_tensor("embT", [emb, batch], F32, kind="ExternalInput")
    emb_d = nc.dram_tensor("embn", [batch, emb], F32, kind="ExternalInput")
    wlab_d = nc.dram_tensor("wlab", [batch, emb], F32, kind="ExternalInput")
    wt_d = nc.dram_tensor("wt", [emb, cpc], F32, kind="ExternalInput")
    out_d = nc.dram_tensor("out", [batch, cpc], F32, kind="ExternalOutput")
    loss_d = nc.dram_tensor("loss", [1, 1], F32, kind="ExternalOutput")
    dbg_d = None
    if debug_outs:
        IC_ = batch // P
        dbg_d = nc.dram_tensor("dbg", [P, IC_ * 6 + 1], F32, kind="ExternalOutput")

    AF = mybir.ActivationFunctionType
    OP = mybir.AluOpType

    with tile.TileContext(nc) as tc:
        with (
            tc.tile_pool(name="const", bufs=1) as constp,
            tc.tile_pool(name="stg", bufs=2) as stgp,
            tc.tile_pool(name="wbf", bufs=1) as wbfp,
            tc.tile_pool(name="ebf", bufs=1) as ebfp,
            tc.tile_pool(name="scale", bufs=1) as scalep,
            tc.tile_pool(name="prep", bufs=2) as prepp,
            tc.tile_pool(name="small", bufs=1) as smallp,
            tc.tile_pool(name="md", bufs=2) as mdp,
            tc.tile_pool(name="obuf", bufs=2) as obufp,
            tc.tile_pool(name="junk", bufs=2) as junkp,
            tc.tile_pool(name="pmm", bufs=2, space="PSUM") as pmmp,
            tc.tile_pool(name="prow", bufs=2, space="PSUM") as prowp,
            tc.tile_pool(name="dram", bufs=1, space="DRAM") as dramp,
        ):
            # ---- constants ----
            identity = constp.tile([P, P], F32)
            make_identity(nc, identity[:, :])
            ones = constp.tile([P, 1], F32)
            nc.vector.memset(ones[:, :], 1.0)
            bias_m64 = constp.tile([P, 1], F32)
            nc.vector.memset(bias_m64[:, :], -float(SCALE))

            # ---- persistent tiles ----
            wt_bf = wbfp.tile([P, KC, cpc], mm_dt)
            embT_bf = ebfp.tile([P, KC, batch], mm_dt)
            scale_bcast = scalep.tile([P, cpc], F32)
            expsums = smallp.tile([P, IC * NG], F32, tag="expsums")
            rawdot = smallp.tile([P, IC], F32, tag="rawdot")
            sumsqE = smallp.tile([P, IC], F32, tag="sumsqE")
            sumsqW = smallp.tile([P, IC], F32, tag="sumsqW")

            # ---- stage embT, convert to matmul dtype ----
            stg_e = stgp.tile([P, KC, batch], F32, tag="stg")
            for k in range(KC):
                nc.sync.dma_start(stg_e[:, k, :], embT_d[k * P:(k + 1) * P, :])
            for k in range(KC):
                nc.scalar.activation(embT_bf[:, k, :], stg_e[:, k, :], AF.Copy)

            # ---- margin/target path: stream emb & wlab natural-layout ----
            junk_pr = smallp.tile([P, emb], F32, tag="junkpr")
            for c in range(IC):
                e_t = prepp.tile([P, emb], F32, tag="ept")
                w_t = prepp.tile([P, emb], F32, tag="wpt")
                nc.scalar.dma_start(e_t[:, :], emb_d[c * P:(c + 1) * P, :])
                nc.scalar.dma_start(w_t[:, :], wlab_d[c * P:(c + 1) * P, :])
                nc.vector.scalar_tensor_tensor(
                    junk_pr[:, :], e_t[:, :], 1.0, w_t[:, :],
                    op0=OP.mult, op1=OP.mult, accum_out=rawdot[:, c:c + 1])
                nc.vector.scalar_tensor_tensor(
                    junk_pr[:, :], e_t[:, :], 1.0, e_t[:, :],
                    op0=OP.mult, op1=OP.mult, accum_out=sumsqE[:, c:c + 1])
                nc.vector.scalar_tensor_tensor(
                    junk_pr[:, :], w_t[:, :], 1.0, w_t[:, :],
                    op0=OP.mult, op1=OP.mult, accum_out=sumsqW[:, c:c + 1])

            # rnorm = 1/||.|| per batch row (per-partition layout)
            tmpE = smallp.tile([P, IC], F32, tag="tmpE")
            rnormE = smallp.tile([P, IC], F32, tag="rnormE")
            rnormE64 = smallp.tile([P, IC], F32, tag="rnormE64")
            nc.vector.reciprocal(tmpE[:, :], sumsqE[:, :])
            nc.scalar.activation(rnormE[:, :], tmpE[:, :], AF.Sqrt)
            nc.scalar.activation(rnormE64[:, :], tmpE[:, :], AF.Sqrt,
                                 scale=float(SCALE) * float(SCALE))
            tmpW = smallp.tile([P, IC], F32, tag="tmpW")
            rnormW = smallp.tile([P, IC], F32, tag="rnormW")
            nc.vector.reciprocal(tmpW[:, :], sumsqW[:, :])
            nc.scalar.activation(rnormW[:, :], tmpW[:, :], AF.Sqrt)

            # target cosine + ArcFace margin (all (P, IC), f32)
            ct = smallp.tile([P, IC], F32, tag="ct")
            nc.vector.tensor_mul(ct[:, :], rawdot[:, :], rnormE[:, :])
            nc.vector.tensor_mul(ct[:, :], ct[:, :], rnormW[:, :])
            nc.vector.tensor_scalar(ct[:, :], ct[:, :], 1.0 - EPS,
                                    -(1.0 - EPS), op0=OP.min, op1=OP.max)
            ct2 = smallp.tile([P, IC], F32, tag="ct2")
            nc.vector.tensor_mul(ct2[:, :], ct[:, :], ct[:, :])
            nc.vector.tensor_scalar(ct2[:, :], ct2[:, :], -1.0, 1.0,
                                    op0=OP.mult, op1=OP.add)  # 1 - ct^2
            sine = smallp.tile([P, IC], F32, tag="sine")
            nc.scalar.activation(sine[:, :], ct2[:, :], AF.Sqrt)
            nc.vector.tensor_scalar_max(sine[:, :], sine[:, :], EPS)
            t1 = smallp.tile([P, IC], F32, tag="t1")
            nc.vector.tensor_scalar_mul(t1[:, :], sine[:, :], SIN_M)
            phi = smallp.tile([P, IC], F32, tag="phi")
            nc.vector.scalar_tensor_tensor(phi[:, :], ct[:, :], COS_M, t1[:, :],
                                           op0=OP.mult, op1=OP.subtract)
            mask = smallp.tile([P, IC], F32, tag="mask")
            nc.vector.tensor_scalar(mask[:, :], ct[:, :], TH, None, op0=OP.is_gt)
            ctmm = smallp.tile([P, IC], F32, tag="ctmm")
            nc.vector.tensor_scalar(ctmm[:, :], ct[:, :], MM_, None,
                                    op0=OP.subtract)
            tl = smallp.tile([P, IC], F32, tag="tl")
            diff = smallp.tile([P, IC], F32, tag="diff")
            nc.vector.tensor_sub(diff[:, :], phi[:, :], ctmm[:, :])
            nc.vector.tensor_mul(diff[:, :], diff[:, :], mask[:, :])
            nc.vector.tensor_add(tl[:, :], ctmm[:, :], diff[:, :])
            # sum-exp correction: replace exp(64ct-64) by exp(64tl-64), /8
            ect = smallp.tile([P, IC], F32, tag="ect")
            nc.scalar.activation(ect[:, :], ct[:, :], AF.Exp,
                                 bias=bias_m64[:, :], scale=float(SCALE))
            etl = smallp.tile([P, IC], F32, tag="etl")
            nc.scalar.activation(etl[:, :], tl[:, :], AF.Exp,
                                 bias=bias_m64[:, :], scale=float(SCALE))
            corr = smallp.tile([P, IC], F32, tag="corr")
            nc.vector.tensor_sub(corr[:, :], ect[:, :], etl[:, :])

            # ---- W^T shard: per-group stage/normalize/matmul pipeline ----
            for g, (js, gw) in enumerate(groups):
                stg_w = stgp.tile([P, KC, gw], F32, tag="stg")
                for k in range(KC):
                    nc.sync.dma_start(stg_w[:, k, :],
                                      wt_d[k * P:(k + 1) * P, js:js + gw])
                for k in range(KC):
                    nc.vector.tensor_copy(wt_bf[:, k, js:js + gw],
                                          stg_w[:, k, :])
                # per-class ||w_j||: square + ones-matmul partition reduce,
                # then sqrt; the divide is fused into the PSUM copy below
                nsq_g = junkp.tile([1, GROUP_JT * JT], F32, tag="nsqg")
                a = 0
                while a < gw:
                    w = min(JT, gw - a)
                    pr = prowp.tile([1, JT], F32, tag="pr")
                    for k in range(KC):
                        w2 = mdp.tile([P, JT], F32, tag="md")
                        nc.vector.tensor_mul(w2[:, :w], stg_w[:, k, a:a + w],
                                             stg_w[:, k, a:a + w])
                        nc.tensor.matmul(pr[:1, :w], ones[:, :], w2[:, :w],
                                         start=(k == 0), stop=(k == KC - 1))
                    nc.scalar.activation(nsq_g[0:1, a:a + w], pr[:1, :w],
                                         AF.Sqrt)
                    a += w
                nc.gpsimd.partition_broadcast(scale_bcast[:, js:js + gw],
                                              nsq_g[0:1, :gw])
                nc.vector.reciprocal(scale_bcast[:, js:js + gw],
                                     scale_bcast[:, js:js + gw])

                # ---- main loop for this group ----
                for ic in range(IC):
                    pt = pmmp.tile([P, GROUP_JT * JT], F32, tag="pt")
                    for k in range(KC):
                        a = 0
                        while a < gw:
                            w = min(JT, gw - a)
                            nc.tensor.matmul(
                                pt[:, a:a + w],
                                embT_bf[:, k, ic * P:(ic + 1) * P],
                                wt_bf[:, k, js + a:js + a + w],
                                start=(k == 0), stop=(k == KC - 1))
                            a += w
                    ob = obufp.tile([P, GROUP_JT * JT], F32, tag="ob")
                    nc.vector.scalar_tensor_tensor(
                        ob[:, :gw], pt[:, :gw], rnormE64[:, ic:ic + 1],
                        scale_bcast[:, js:js + gw], op0=OP.mult,
                        op1=OP.mult)
                    nc.sync.dma_start(out_d[ic * P:(ic + 1) * P, js:js + gw],
                                      ob[:, :gw])
                    je = junkp.tile([P, GROUP_JT * JT], F32, tag="je")
                    nc.scalar.activation(je[:, :gw], ob[:, :gw], AF.Exp,
                                         bias=bias_m64[:, :],
                                         accum_out=expsums[:, ic * NG + g:
                                                           ic * NG + g + 1])

            # ---- softmax denominator: local reduce, correct, AllReduce ----
            lsum = smallp.tile([P, IC], F32, tag="lsum")
            for ic in range(IC):
                nc.vector.tensor_reduce(lsum[:, ic:ic + 1],
                                        expsums[:, ic * NG:(ic + 1) * NG],
                                        axis=mybir.AxisListType.X, op=OP.add)
            lsumc = smallp.tile([P, IC], F32, tag="lsumc")
            nc.vector.scalar_tensor_tensor(lsumc[:, :], corr[:, :],
                                           -1.0 / n_cores, lsum[:, :],
                                           op0=OP.mult, op1=OP.add)

            cc_in = dramp.tile([P, IC], F32, tag="ccin")
            cc_out = dramp.tile([P, IC], F32, tag="ccout")
            nc.sync.dma_start(cc_in[:, :], lsumc[:, :])
            nc.gpsimd.collective_compute(
                "AllReduce", OP.add,
                replica_groups=[list(range(n_cores))],
                ins=[cc_in.opt()], outs=[cc_out.opt()])
            gsum = smallp.tile([P, IC], F32, tag="gsum")
            nc.sync.dma_start(gsum[:, :], cc_out[:, :])

            # loss = mean(64 + ln(gsum) - 64*tl)
            # Ln is inaccurate for tiny inputs: rescale into ~[1e-2, 1e2]
            # via ln(g) = ln(g * 2^64) - 64*ln2 (the -64*ln2 lands in the
            # final Copy bias below).
            lg = smallp.tile([P, IC], F32, tag="lg")
            nc.scalar.activation(lg[:, :], gsum[:, :], AF.Ln,
                                 scale=float(2.0 ** 64))
            nll = smallp.tile([P, IC], F32, tag="nll")
            nc.vector.scalar_tensor_tensor(nll[:, :], tl[:, :], -float(SCALE),
                                           lg[:, :], op0=OP.mult, op1=OP.add)
            nllr = smallp.tile([P, 1], F32, tag="nllr")
            nc.vector.tensor_reduce(nllr[:, :], nll[:, :],
                                    axis=mybir.AxisListType.X, op=OP.add)
            pl = prowp.tile([1, P], F32, tag="pr")
            nc.tensor.matmul(pl[:1, 0:1], ones[:, :], nllr[:, :],
                             start=True, stop=True)
            loss_sb = smallp.tile([1, 1], F32, tag="losssb")
            nc.scalar.activation(loss_sb[:, :], pl[:1, 0:1], AF.Copy,
                                 bias=float(SCALE) - 64.0 * math.log(2.0),
                                 scale=1.0 / batch)
            nc.sync.dma_start(loss_d[:, :], loss_sb[:, :])
            if debug_outs:
                nc.sync.dma_start(dbg_d[:, 0:IC], lsumc[:, :])
                nc.sync.dma_start(dbg_d[:, IC:2 * IC], gsum[:, :])
                nc.sync.dma_start(dbg_d[:, 2 * IC:3 * IC], ct[:, :])
                nc.sync.dma_start(dbg_d[:, 3 * IC:4 * IC], tl[:, :])
                nc.sync.dma_start(dbg_d[:, 4 * IC:5 * IC], lg[:, :])
                nc.sync.dma_start(dbg_d[:, 5 * IC:6 * IC], nll[:, :])
                nc.sync.dma_start(dbg_d[:, 6 * IC:6 * IC + 1], nllr[:, :])

    nc.compile()
    return nc


def _host_inputs(emb, label, W, cpc, n_cores):
    emb = np.ascontiguousarray(np.asarray(emb, dtype=np.float32))
    W = np.ascontiguousarray(np.asarray(W, dtype=np.float32))
    label = np.asarray(label).astype(np.int64)
    embT = np.ascontiguousarray(emb.T)
    wlab = np.ascontiguousarray(W[label])
    WT = np.ascontiguousarray(W.T)
    in_maps = []
    for d in range(n_cores):
        in_maps.append({
            "embT": embT,
            "embn": emb,
            "wlab": wlab,
            "wt": np.ascontiguousarray(WT[:, d * cpc:(d + 1) * cpc]),
        })
    return in_maps


_NC_CACHE = {}
LAST_RESULTS = None


def kernel(emb, label, W):
    global LAST_RESULTS
    from concourse.bass_utils import run_bass_kernel_spmd

    key = "full"
    if key not in _NC_CACHE:
        _NC_CACHE[key] = build_nc()
    nc = _NC_CACHE[key]

    in_maps = _host_inputs(emb, label, W, CPC, N_CORES)
    res = run_bass_kernel_spmd(nc, in_maps, core_ids=list(range(N_CORES)))
    LAST_RESULTS = res
    logits = np.concatenate([r["out"] for r in res.results], axis=1)
    loss = np.asarray(res.results[0]["loss"][0, 0], dtype=np.float32)
    return loss, logits
